# revision 14
# baseline (speedup 1.0000x reference)
"""GNN NodeModel (message passing + scatter-mean + node MLP) on 8 trn2 cores.

Strategy:
  * Host partitions destination nodes into 8 contiguous ranges with equal edge
    counts; each core processes the edges targeting its range (edges sorted by
    destination on host, so the scatter-sum is core-local -- no collectives).
  * Per core, destinations are grouped into "blocks" of <=128 consecutive
    dests; each block's edge list is padded to a fixed number of 128-edge
    chunks (NCHB per block, uniform across blocks and cores so all 8 cores run
    one SPMD program).
  * The host lays out three sequential bf16 streams in chunk order:
    x[src] (feature-major), edge_attr (feature-major), and the one-hot
    dest-selection matrix for the segment-sum.  All device traffic is
    sequential DMA at full bandwidth.
  * Edge phase per chunk c (128 edges):
        g   = x[src]@W1a + ea@W1b (+b1)          (PE, bf16 in, fp32 PSUM)
        R   = relu(g)                            (ACT/DVE alternating, -> bf16)
        S_b += onehot^T @ R                      (PE; segment-sum as matmul)
  * Block tail: S_mean = S_b * inv_count (DVE), PE-transpose to feature-major.
  * Node phase per 4 blocks (512 dests): feature-major MLP2 with host-folded
    weights (W2 @ W3_mid), u[batch] and the count>0 flag baked into a table.
  * Output returned feature-major per core; host scatters back to [N, 128].
"""

import sys
import types

import numpy as np
import ml_dtypes

BF16 = ml_dtypes.bfloat16
SLAB = 4096  # stream columns per DMA
NCORES = 8
ACT_RELU_FRAC = 0.4  # fraction of relu pair-ops on ScalarE (rest on VectorE)

_CACHE = {}


def _cut_blocks(d0, d1, cnt, cap):
    """Greedy block cut: <=128 consecutive dests, <=cap edges per block."""
    blocks = []
    d = d0
    while d < d1:
        b0 = d
        tot = 0
        while d < d1 and (d - b0) < 128 and tot + cnt[d] <= cap:
            tot += cnt[d]
            d += 1
        if d == b0:
            return None
        blocks.append((b0, d))
    return blocks


def _host_prep(inputs):
    x = np.asarray(inputs["x"], np.float32)
    ei = np.asarray(inputs["edge_index"])
    ea = np.asarray(inputs["edge_attr"], np.float32)
    u = np.asarray(inputs["u"], np.float32)
    batch = np.asarray(inputs["batch"]).astype(np.int64)
    W1 = np.asarray(inputs["W1"], np.float32)
    b1 = np.asarray(inputs["b1"], np.float32)
    W2 = np.asarray(inputs["W2"], np.float32)
    b2 = np.asarray(inputs["b2"], np.float32)
    W3 = np.asarray(inputs["W3"], np.float32)
    b3 = np.asarray(inputs["b3"], np.float32)
    W4 = np.asarray(inputs["W4"], np.float32)
    b4 = np.asarray(inputs["b4"], np.float32)

    N, NF = x.shape
    E, EF = ea.shape
    G, UF = u.shape
    H = W1.shape[1]
    FOUT = W4.shape[1]
    assert NF == 128 and EF == 128 and H == 256 and UF + 1 <= 128 and FOUT == 128

    row = ei[0].astype(np.int64)
    col = ei[1].astype(np.int64)

    order = np.argsort(col, kind="stable")
    cnt = np.bincount(col, minlength=N)
    ends = np.cumsum(cnt)
    starts = ends - cnt

    bounds = [0]
    for k in range(1, NCORES):
        bounds.append(int(np.searchsorted(ends, round(E * k / NCORES))))
    bounds.append(N)

    # pick NCHB (chunks per block, even) minimizing padded slot count
    best = None
    for C in range(2, 24, 2):
        per_core = []
        ok = True
        for k in range(NCORES):
            blks = _cut_blocks(bounds[k], bounds[k + 1], cnt, C * 128)
            if blks is None:
                ok = False
                break
            per_core.append(blks)
        if not ok:
            continue
        nb = max(len(b) for b in per_core)
        cost = nb * C
        if best is None or cost < best[0]:
            best = (cost, C, nb, per_core)
    assert best is not None
    _, NCHB, NB, per_core_blocks = best
    NB = -(-NB // 4) * 4
    NCH = NB * NCHB

    meta = dict(N=N, NF=NF, EF=EF, H=H, UF=UF, FOUT=FOUT, NB=NB, NCHB=NCHB,
                NCH=NCH, has_b1=bool(b1.any()))

    in_maps = []
    dest_maps = []
    for k in range(NCORES):
        blocks = per_core_blocks[k]
        eid = np.zeros(NCH * 128, np.int64)
        colv = np.full(NCH * 128, -1, np.int64)
        dest_of_slot = np.full(NB * 128, -1, np.int64)
        for b, (d0, d1) in enumerate(blocks):
            ids = order[starts[d0]:ends[d1 - 1]]
            n = len(ids)
            ubase = b * NCHB * 128
            eid[ubase:ubase + n] = ids
            colv[ubase:ubase + n] = col[ids] - d0
            dest_of_slot[b * 128:b * 128 + (d1 - d0)] = np.arange(d0, d1)

        # one-hot stream: oh[e_part, ch*128 + d] = (col_adj(slot) == d)
        oh = np.zeros((128, NCH * 128), BF16)
        q = np.nonzero(colv >= 0)[0]
        oh[q % 128, (q // 128) * 128 + colv[q]] = 1

        valid = dest_of_slot >= 0
        xt = np.zeros((NF, NB * 128), BF16)
        xt[:, valid] = x[dest_of_slot[valid]].T.astype(BF16)
        ut = np.zeros((128, NB * 128), BF16)
        ut[:UF, valid] = u[batch[dest_of_slot[valid]]].T.astype(BF16)
        ut[UF, valid] = (cnt[dest_of_slot[valid]] > 0).astype(BF16)
        inv = np.where(valid, 1.0 / np.maximum(cnt[dest_of_slot.clip(0)], 1),
                       0.0).astype(np.float32)
        inv = np.ascontiguousarray(inv.reshape(NB, 128).T)

        m = dict(
            xg_t=np.ascontiguousarray(x[row[eid]].astype(BF16).T),
            ea_t=np.ascontiguousarray(ea[eid].astype(BF16).T),
            oh_t=oh,
            x_t=xt,
            u_t=ut,
            inv=inv,
        )
        in_maps.append(m)
        dest_maps.append(dest_of_slot)

    W2W3 = (W2 @ W3[NF:NF + H]).astype(np.float32)
    ns3 = np.zeros((128, H), np.float32)
    ns3[:UF] = W3[NF + H:NF + H + UF]
    ns3[UF] = b2 @ W3[NF:NF + H]
    shared = dict(
        w1a=np.ascontiguousarray(W1[:NF].astype(BF16)),
        w1b=np.ascontiguousarray(W1[NF:].astype(BF16)),
        ns0=np.ascontiguousarray(W3[:NF].astype(BF16)),
        ns1=np.ascontiguousarray(W2W3[:128].astype(BF16)),
        ns2=np.ascontiguousarray(W2W3[128:].astype(BF16)),
        ns3=ns3.astype(BF16),
        w4s=np.ascontiguousarray(
            np.concatenate([W4[:128], W4[128:]], axis=1).astype(BF16)),
        b3t=np.stack([b3[:128], b3[128:]], axis=1).astype(np.float32),
        b4t=b4.reshape(128, 1).astype(np.float32),
        ident=np.eye(128, dtype=np.float32),
    )
    if meta["has_b1"]:
        shared["b1r"] = np.tile(b1.astype(BF16)[None, :], (128, 1))
        shared["ones1"] = np.ones((128, 128), BF16)
    for m in in_maps:
        m.update(shared)
    return meta, in_maps, dest_maps


def _build_program(meta):
    import concourse.bacc as bacc
    import concourse.mybir as mybir
    import concourse.tile as tile

    f32 = mybir.dt.float32
    bf16 = mybir.dt.bfloat16
    AF = mybir.ActivationFunctionType
    ALU = mybir.AluOpType

    NF, H = meta["NF"], meta["H"]
    NB, NCHB, NCH = meta["NB"], meta["NCHB"], meta["NCH"]
    has_b1 = meta["has_b1"]
    ND = NB * 128
    NG = ND // 512

    nc = bacc.Bacc("TRN2", target_bir_lowering=False, debug=False,
                   num_devices=NCORES)

    def din(name, shape, dt):
        return nc.dram_tensor(name, shape, dt, kind="ExternalInput")

    xg_t = din("xg_t", [128, NCH * 128], bf16)
    ea_t = din("ea_t", [128, NCH * 128], bf16)
    oh_t = din("oh_t", [128, NCH * 128], bf16)
    x_t = din("x_t", [128, ND], bf16)
    u_t = din("u_t", [128, ND], bf16)
    inv = din("inv", [128, NB], f32)
    w1a = din("w1a", [128, H], bf16)
    w1b = din("w1b", [128, H], bf16)
    ns0 = din("ns0", [128, H], bf16)
    ns1 = din("ns1", [128, H], bf16)
    ns2 = din("ns2", [128, H], bf16)
    ns3 = din("ns3", [128, H], bf16)
    w4s = din("w4s", [128, 256], bf16)
    b3t = din("b3t", [128, 2], f32)
    b4t = din("b4t", [128, 1], f32)
    ident = din("ident", [128, 128], f32)
    if has_b1:
        b1r = din("b1r", [128, H], bf16)
        ones1 = din("ones1", [128, 128], bf16)
    out_t = nc.dram_tensor("out_t", [128, ND], f32, kind="ExternalOutput")

    with tile.TileContext(nc) as tc:
        from contextlib import ExitStack

        with ExitStack() as ctx:
            cpool = ctx.enter_context(tc.tile_pool(name="const", bufs=1))
            gpool = ctx.enter_context(tc.tile_pool(name="slabs", bufs=3))
            wpool = ctx.enter_context(tc.tile_pool(name="work", bufs=6))
            spool = ctx.enter_context(tc.tile_pool(name="smean", bufs=2))
            z1pool = ctx.enter_context(tc.tile_pool(name="z1", bufs=2))
            opool = ctx.enter_context(tc.tile_pool(name="osb", bufs=2))
            pp3 = ctx.enter_context(
                tc.tile_pool(name="pp3", bufs=3, space="PSUM"))
            pp2 = ctx.enter_context(
                tc.tile_pool(name="pp2", bufs=2, space="PSUM"))
            pp1 = ctx.enter_context(
                tc.tile_pool(name="pp1", bufs=1, space="PSUM"))

            def load_const(dram, shape, dt, tag, eng=None):
                t = cpool.tile(shape, dt, tag=tag)
                (eng or nc.sync).dma_start(t[:], dram.ap())
                return t

            slab_state = {"xg": (-1, None), "ea": (-1, None), "oh": (-1, None)}
            streams = {"xg": xg_t, "ea": ea_t, "oh": oh_t}

            def get_slab(kind, idx):
                cur, t = slab_state[kind]
                if cur != idx:
                    t = gpool.tile([128, SLAB], bf16, tag=kind)
                    w = min(SLAB, NCH * 128 - idx * SLAB)
                    nc.sync.dma_start(
                        t[:, 0:w],
                        streams[kind].ap()[:, idx * SLAB:idx * SLAB + w])
                    slab_state[kind] = (idx, t)
                return t

            # weights needed by the first matmuls, then stream prefetch
            w1a_sb = load_const(w1a, [128, H], bf16, "w1a")
            w1b_sb = load_const(w1b, [128, H], bf16, "w1b")
            for kind in ("xg", "ea", "oh"):
                get_slab(kind, 0)
            # node-phase constants ride the ACT HWDGE ring (parallel to sync)
            ident_sb = load_const(ident, [128, 128], f32, "ident", nc.scalar)
            ns_sb = [load_const(t, [128, H], bf16, f"ns{i}", nc.scalar)
                     for i, t in enumerate([ns0, ns1, ns2, ns3])]
            w4_sb = load_const(w4s, [128, 256], bf16, "w4s", nc.scalar)
            b3_sb = load_const(b3t, [128, 2], f32, "b3t", nc.scalar)
            b4_sb = load_const(b4t, [128, 1], f32, "b4t", nc.scalar)
            inv_sb = load_const(inv, [128, NB], f32, "inv", nc.scalar)
            # node tables are loaded lazily (deferred past the first blocks
            # so the edge-stream slabs own the DMA lanes at kernel start)
            xt_sb = cpool.tile([128, ND], bf16, tag="xt")
            ut_sb = cpool.tile([128, ND], bf16, tag="ut")
            node_tables = [False]

            def ensure_node_tables():
                if not node_tables[0]:
                    node_tables[0] = True
                    nc.scalar.dma_start(xt_sb[:], x_t.ap())
                    nc.scalar.dma_start(ut_sb[:], u_t.ap())
            if has_b1:
                b1r_sb = load_const(b1r, [128, H], bf16, "b1r", nc.scalar)
                ones_sb = load_const(ones1, [128, 128], bf16, "ones1",
                                     nc.scalar)
            s1_sb = cpool.tile([128, ND], bf16, tag="s1")
            s2_sb = cpool.tile([128, ND], bf16, tag="s2")

            def chunk_slice(kind, ch):
                t = get_slab(kind, ch // (SLAB // 128))
                c = (ch % (SLAB // 128)) * 128
                return t[:, c:c + 128]

            def node_group(g):
                ensure_node_tables()
                sl = slice(g * 512, (g + 1) * 512)
                zh = []
                for hh in range(2):
                    z = pp1.tile([128, 512], f32, tag=f"z{hh}")
                    ml = slice(hh * 128, (hh + 1) * 128)
                    nc.tensor.matmul(z[:], ns_sb[0][:, ml], xt_sb[:, sl],
                                     start=True, stop=False)
                    nc.tensor.matmul(z[:], ns_sb[1][:, ml], s1_sb[:, sl],
                                     start=False, stop=False)
                    nc.tensor.matmul(z[:], ns_sb[2][:, ml], s2_sb[:, sl],
                                     start=False, stop=False)
                    nc.tensor.matmul(z[:], ns_sb[3][:, ml], ut_sb[:, sl],
                                     start=False, stop=True)
                    zb = z1pool.tile([128, 512], bf16, tag=f"z1b{hh}")
                    nc.scalar.activation(zb[:], z[:], AF.Relu,
                                         bias=b3_sb[:, hh:hh + 1])
                    zh.append(zb)
                o = pp1.tile([128, 512], f32, tag="misc")
                nc.tensor.matmul(o[:], w4_sb[:, 0:128], zh[0][:],
                                 start=True, stop=False)
                nc.tensor.matmul(o[:], w4_sb[:, 128:256], zh[1][:],
                                 start=False, stop=True)
                osb = opool.tile([128, 512], f32, tag="osb")
                nc.scalar.activation(osb[:], o[:], AF.Identity,
                                     bias=b4_sb[:, 0:1])
                nc.sync.dma_start(out_t.ap()[:, sl], osb[:])

            relu_acc = 0.0
            for b in range(NB):
                S = pp2.tile([128, H], f32, tag="S")
                for jj in range(0, NCHB, 2):
                    g2 = pp3.tile([128, 512], f32, tag="g")
                    for t in (0, 1):
                        ch = b * NCHB + jj + t
                        gs = g2[:, t * 256:(t + 1) * 256]
                        nc.tensor.matmul(gs, chunk_slice("xg", ch), w1a_sb[:],
                                         start=True, stop=False)
                        nc.tensor.matmul(gs, chunk_slice("ea", ch), w1b_sb[:],
                                         start=False, stop=not has_b1)
                        if has_b1:
                            nc.tensor.matmul(gs, ones_sb[0:1, :],
                                             b1r_sb[0:1, :],
                                             start=False, stop=True)
                    r2 = wpool.tile([128, 512], bf16, tag="r")
                    relu_acc += ACT_RELU_FRAC
                    if relu_acc >= 1.0:
                        relu_acc -= 1.0
                        nc.scalar.activation(r2[:], g2[:], AF.Relu)
                    else:
                        nc.vector.tensor_scalar(r2[:], g2[:], 0.0, None,
                                                ALU.max)
                    for t in (0, 1):
                        ch = b * NCHB + jj + t
                        nc.tensor.matmul(S[:], chunk_slice("oh", ch),
                                         r2[:, t * 256:(t + 1) * 256],
                                         start=(jj + t == 0),
                                         stop=(jj + t == NCHB - 1))
                # block tail
                sm = spool.tile([128, H], f32, tag="sm")
                nc.vector.tensor_scalar_mul(sm[:], S[:], inv_sb[:, b:b + 1])
                tr = pp1.tile([128, H], f32, tag="misc")
                nc.tensor.transpose(tr[:, 0:128], sm[:, 0:128], ident_sb[:])
                nc.tensor.transpose(tr[:, 128:256], sm[:, 128:256],
                                    ident_sb[:])
                nc.scalar.activation(s1_sb[:, b * 128:(b + 1) * 128],
                                     tr[:, 0:128], AF.Copy)
                nc.scalar.activation(s2_sb[:, b * 128:(b + 1) * 128],
                                     tr[:, 128:256], AF.Copy)
                if (b + 1) % 4 == 0:
                    node_group((b + 1) // 4 - 1)

    nc.compile()
    return nc


# ------------------------------------------------------------------- driver

def _install_ntff_hook():
    try:
        import antenv
        import concourse.bass_utils as bass_utils
        import trn_agent_boot.trn_boot as tb

        if "antenv.axon_hooks" in sys.modules:
            return
        mod = types.ModuleType("antenv.axon_hooks")
        holder = [None]
        mod.set_axon_ntff_profile_hook = lambda h: holder.__setitem__(0, h)
        mod.get_axon_ntff_profile_hook = lambda: holder[0]
        sys.modules["antenv.axon_hooks"] = mod
        antenv.axon_hooks = mod
        mod.set_axon_ntff_profile_hook(
            tb._ntff_profile_via_ctypes("/opt/axon/libaxon_pjrt.so"))
        bass_utils.upload_artifacts = lambda tmpdir: tmpdir
    except Exception:
        pass


def _run(inputs, trace=False, trace_cores=None):
    from concourse.bass_utils import run_bass_kernel_spmd

    if trace:
        _install_ntff_hook()
    meta, in_maps, dest_maps = _host_prep(inputs)
    key = tuple(sorted(meta.items()))
    if key not in _CACHE:
        _CACHE[key] = _build_program(meta)
    nc = _CACHE[key]
    res = run_bass_kernel_spmd(nc, in_maps, list(range(NCORES)), trace=trace,
                               trace_cores=trace_cores)
    N, FOUT = meta["N"], meta["FOUT"]
    out = np.zeros((N, FOUT), np.float32)
    for k in range(NCORES):
        o = res.results[k]["out_t"]
        dm = dest_maps[k]
        valid = dm >= 0
        out[dm[valid]] = o.T[valid]
    return out, res


def kernel(**inputs):
    return _run(inputs, trace=False)[0]


# revision 15
# speedup vs baseline: 1.0163x; 1.0163x over previous
"""GNN NodeModel (message passing + scatter-mean + node MLP) on 8 trn2 cores.

Strategy:
  * Host partitions destination nodes into 8 contiguous ranges with equal edge
    counts; each core processes the edges targeting its range (edges sorted by
    destination on host, so the scatter-sum is core-local -- no collectives).
  * Per core, destinations are grouped into "blocks" of <=128 consecutive
    dests; each block's edge list is padded to a fixed number of 128-edge
    chunks (NCHB per block, uniform across blocks and cores so all 8 cores run
    one SPMD program).
  * The host lays out three sequential bf16 streams in chunk order:
    x[src] (feature-major), edge_attr (feature-major), and the one-hot
    dest-selection matrix for the segment-sum.  All device traffic is
    sequential DMA at full bandwidth.
  * Edge phase per chunk c (128 edges):
        g   = x[src]@W1a + ea@W1b (+b1)          (PE, bf16 in, fp32 PSUM)
        R   = relu(g)                            (ACT/DVE alternating, -> bf16)
        S_b += onehot^T @ R                      (PE; segment-sum as matmul)
  * Block tail: S_mean = S_b * inv_count (DVE), PE-transpose to feature-major.
  * Node phase per 4 blocks (512 dests): feature-major MLP2 with host-folded
    weights (W2 @ W3_mid), u[batch] and the count>0 flag baked into a table.
  * Output returned feature-major per core; host scatters back to [N, 128].
"""

import sys
import types

import numpy as np
import ml_dtypes

BF16 = ml_dtypes.bfloat16
SLAB = 2048  # stream columns per DMA
NCORES = 8
ACT_RELU_FRAC = 0.4  # fraction of relu pair-ops on ScalarE (rest on VectorE)

_CACHE = {}


def _cut_blocks(d0, d1, cnt, cap):
    """Greedy block cut: <=128 consecutive dests, <=cap edges per block."""
    blocks = []
    d = d0
    while d < d1:
        b0 = d
        tot = 0
        while d < d1 and (d - b0) < 128 and tot + cnt[d] <= cap:
            tot += cnt[d]
            d += 1
        if d == b0:
            return None
        blocks.append((b0, d))
    return blocks


def _host_prep(inputs):
    x = np.asarray(inputs["x"], np.float32)
    ei = np.asarray(inputs["edge_index"])
    ea = np.asarray(inputs["edge_attr"], np.float32)
    u = np.asarray(inputs["u"], np.float32)
    batch = np.asarray(inputs["batch"]).astype(np.int64)
    W1 = np.asarray(inputs["W1"], np.float32)
    b1 = np.asarray(inputs["b1"], np.float32)
    W2 = np.asarray(inputs["W2"], np.float32)
    b2 = np.asarray(inputs["b2"], np.float32)
    W3 = np.asarray(inputs["W3"], np.float32)
    b3 = np.asarray(inputs["b3"], np.float32)
    W4 = np.asarray(inputs["W4"], np.float32)
    b4 = np.asarray(inputs["b4"], np.float32)

    N, NF = x.shape
    E, EF = ea.shape
    G, UF = u.shape
    H = W1.shape[1]
    FOUT = W4.shape[1]
    assert NF == 128 and EF == 128 and H == 256 and UF + 1 <= 128 and FOUT == 128

    row = ei[0].astype(np.int64)
    col = ei[1].astype(np.int64)

    order = np.argsort(col, kind="stable")
    cnt = np.bincount(col, minlength=N)
    ends = np.cumsum(cnt)
    starts = ends - cnt

    bounds = [0]
    for k in range(1, NCORES):
        bounds.append(int(np.searchsorted(ends, round(E * k / NCORES))))
    bounds.append(N)

    # pick NCHB (chunks per block, even) minimizing padded slot count
    best = None
    for C in range(2, 24, 2):
        per_core = []
        ok = True
        for k in range(NCORES):
            blks = _cut_blocks(bounds[k], bounds[k + 1], cnt, C * 128)
            if blks is None:
                ok = False
                break
            per_core.append(blks)
        if not ok:
            continue
        nb = max(len(b) for b in per_core)
        cost = nb * C
        if best is None or cost < best[0]:
            best = (cost, C, nb, per_core)
    assert best is not None
    _, NCHB, NB, per_core_blocks = best
    NB = -(-NB // 4) * 4
    NCH = NB * NCHB

    meta = dict(N=N, NF=NF, EF=EF, H=H, UF=UF, FOUT=FOUT, NB=NB, NCHB=NCHB,
                NCH=NCH, has_b1=bool(b1.any()))

    in_maps = []
    dest_maps = []
    for k in range(NCORES):
        blocks = per_core_blocks[k]
        eid = np.zeros(NCH * 128, np.int64)
        colv = np.full(NCH * 128, -1, np.int64)
        dest_of_slot = np.full(NB * 128, -1, np.int64)
        for b, (d0, d1) in enumerate(blocks):
            ids = order[starts[d0]:ends[d1 - 1]]
            n = len(ids)
            ubase = b * NCHB * 128
            eid[ubase:ubase + n] = ids
            colv[ubase:ubase + n] = col[ids] - d0
            dest_of_slot[b * 128:b * 128 + (d1 - d0)] = np.arange(d0, d1)

        # one-hot stream: oh[e_part, ch*128 + d] = (col_adj(slot) == d)
        oh = np.zeros((128, NCH * 128), BF16)
        q = np.nonzero(colv >= 0)[0]
        oh[q % 128, (q // 128) * 128 + colv[q]] = 1

        valid = dest_of_slot >= 0
        xt = np.zeros((NF, NB * 128), BF16)
        xt[:, valid] = x[dest_of_slot[valid]].T.astype(BF16)
        ut = np.zeros((128, NB * 128), BF16)
        ut[:UF, valid] = u[batch[dest_of_slot[valid]]].T.astype(BF16)
        ut[UF, valid] = (cnt[dest_of_slot[valid]] > 0).astype(BF16)
        inv = np.where(valid, 1.0 / np.maximum(cnt[dest_of_slot.clip(0)], 1),
                       0.0).astype(np.float32)
        inv = np.ascontiguousarray(inv.reshape(NB, 128).T)

        m = dict(
            xg_t=np.ascontiguousarray(x[row[eid]].astype(BF16).T),
            ea_t=np.ascontiguousarray(ea[eid].astype(BF16).T),
            oh_t=oh,
            x_t=xt,
            u_t=ut,
            inv=inv,
        )
        in_maps.append(m)
        dest_maps.append(dest_of_slot)

    W2W3 = (W2 @ W3[NF:NF + H]).astype(np.float32)
    ns3 = np.zeros((128, H), np.float32)
    ns3[:UF] = W3[NF + H:NF + H + UF]
    ns3[UF] = b2 @ W3[NF:NF + H]
    shared = dict(
        w1a=np.ascontiguousarray(W1[:NF].astype(BF16)),
        w1b=np.ascontiguousarray(W1[NF:].astype(BF16)),
        ns0=np.ascontiguousarray(W3[:NF].astype(BF16)),
        ns1=np.ascontiguousarray(W2W3[:128].astype(BF16)),
        ns2=np.ascontiguousarray(W2W3[128:].astype(BF16)),
        ns3=ns3.astype(BF16),
        w4s=np.ascontiguousarray(
            np.concatenate([W4[:128], W4[128:]], axis=1).astype(BF16)),
        b3t=np.stack([b3[:128], b3[128:]], axis=1).astype(np.float32),
        b4t=b4.reshape(128, 1).astype(np.float32),
        ident=np.eye(128, dtype=np.float32),
    )
    if meta["has_b1"]:
        shared["b1r"] = np.tile(b1.astype(BF16)[None, :], (128, 1))
        shared["ones1"] = np.ones((128, 128), BF16)
    for m in in_maps:
        m.update(shared)
    return meta, in_maps, dest_maps


def _build_program(meta):
    import concourse.bacc as bacc
    import concourse.mybir as mybir
    import concourse.tile as tile

    f32 = mybir.dt.float32
    bf16 = mybir.dt.bfloat16
    AF = mybir.ActivationFunctionType
    ALU = mybir.AluOpType

    NF, H = meta["NF"], meta["H"]
    NB, NCHB, NCH = meta["NB"], meta["NCHB"], meta["NCH"]
    has_b1 = meta["has_b1"]
    ND = NB * 128
    NG = ND // 512

    nc = bacc.Bacc("TRN2", target_bir_lowering=False, debug=False,
                   num_devices=NCORES)

    def din(name, shape, dt):
        return nc.dram_tensor(name, shape, dt, kind="ExternalInput")

    xg_t = din("xg_t", [128, NCH * 128], bf16)
    ea_t = din("ea_t", [128, NCH * 128], bf16)
    oh_t = din("oh_t", [128, NCH * 128], bf16)
    x_t = din("x_t", [128, ND], bf16)
    u_t = din("u_t", [128, ND], bf16)
    inv = din("inv", [128, NB], f32)
    w1a = din("w1a", [128, H], bf16)
    w1b = din("w1b", [128, H], bf16)
    ns0 = din("ns0", [128, H], bf16)
    ns1 = din("ns1", [128, H], bf16)
    ns2 = din("ns2", [128, H], bf16)
    ns3 = din("ns3", [128, H], bf16)
    w4s = din("w4s", [128, 256], bf16)
    b3t = din("b3t", [128, 2], f32)
    b4t = din("b4t", [128, 1], f32)
    ident = din("ident", [128, 128], f32)
    if has_b1:
        b1r = din("b1r", [128, H], bf16)
        ones1 = din("ones1", [128, 128], bf16)
    out_t = nc.dram_tensor("out_t", [128, ND], f32, kind="ExternalOutput")

    with tile.TileContext(nc) as tc:
        from contextlib import ExitStack

        with ExitStack() as ctx:
            cpool = ctx.enter_context(tc.tile_pool(name="const", bufs=1))
            gpool = ctx.enter_context(tc.tile_pool(name="slabs", bufs=3))
            wpool = ctx.enter_context(tc.tile_pool(name="work", bufs=6))
            spool = ctx.enter_context(tc.tile_pool(name="smean", bufs=2))
            z1pool = ctx.enter_context(tc.tile_pool(name="z1", bufs=2))
            opool = ctx.enter_context(tc.tile_pool(name="osb", bufs=2))
            pp3 = ctx.enter_context(
                tc.tile_pool(name="pp3", bufs=3, space="PSUM"))
            pp2 = ctx.enter_context(
                tc.tile_pool(name="pp2", bufs=2, space="PSUM"))
            pp1 = ctx.enter_context(
                tc.tile_pool(name="pp1", bufs=1, space="PSUM"))

            def load_const(dram, shape, dt, tag, eng=None):
                t = cpool.tile(shape, dt, tag=tag)
                (eng or nc.sync).dma_start(t[:], dram.ap())
                return t

            slab_state = {"xg": (-1, None), "ea": (-1, None), "oh": (-1, None)}
            streams = {"xg": xg_t, "ea": ea_t, "oh": oh_t}

            def get_slab(kind, idx):
                cur, t = slab_state[kind]
                if cur != idx:
                    t = gpool.tile([128, SLAB], bf16, tag=kind)
                    w = min(SLAB, NCH * 128 - idx * SLAB)
                    nc.sync.dma_start(
                        t[:, 0:w],
                        streams[kind].ap()[:, idx * SLAB:idx * SLAB + w])
                    slab_state[kind] = (idx, t)
                return t

            # weights needed by the first matmuls, then stream prefetch
            w1a_sb = load_const(w1a, [128, H], bf16, "w1a")
            w1b_sb = load_const(w1b, [128, H], bf16, "w1b")
            for kind in ("xg", "ea", "oh"):
                get_slab(kind, 0)
            # node-phase constants ride the ACT HWDGE ring (parallel to sync)
            ident_sb = load_const(ident, [128, 128], f32, "ident", nc.scalar)
            ns_sb = [load_const(t, [128, H], bf16, f"ns{i}", nc.scalar)
                     for i, t in enumerate([ns0, ns1, ns2, ns3])]
            w4_sb = load_const(w4s, [128, 256], bf16, "w4s", nc.scalar)
            b3_sb = load_const(b3t, [128, 2], f32, "b3t", nc.scalar)
            b4_sb = load_const(b4t, [128, 1], f32, "b4t", nc.scalar)
            inv_sb = load_const(inv, [128, NB], f32, "inv", nc.scalar)
            # node tables are loaded lazily (deferred past the first blocks
            # so the edge-stream slabs own the DMA lanes at kernel start)
            xt_sb = cpool.tile([128, ND], bf16, tag="xt")
            ut_sb = cpool.tile([128, ND], bf16, tag="ut")
            node_tables = [False]

            def ensure_node_tables():
                if not node_tables[0]:
                    node_tables[0] = True
                    nc.scalar.dma_start(xt_sb[:], x_t.ap())
                    nc.scalar.dma_start(ut_sb[:], u_t.ap())
            if has_b1:
                b1r_sb = load_const(b1r, [128, H], bf16, "b1r", nc.scalar)
                ones_sb = load_const(ones1, [128, 128], bf16, "ones1",
                                     nc.scalar)
            s1_sb = cpool.tile([128, ND], bf16, tag="s1")
            s2_sb = cpool.tile([128, ND], bf16, tag="s2")

            def chunk_slice(kind, ch):
                t = get_slab(kind, ch // (SLAB // 128))
                c = (ch % (SLAB // 128)) * 128
                return t[:, c:c + 128]

            def node_group(g):
                ensure_node_tables()
                sl = slice(g * 512, (g + 1) * 512)
                zh = []
                for hh in range(2):
                    z = pp1.tile([128, 512], f32, tag=f"z{hh}")
                    ml = slice(hh * 128, (hh + 1) * 128)
                    nc.tensor.matmul(z[:], ns_sb[0][:, ml], xt_sb[:, sl],
                                     start=True, stop=False)
                    nc.tensor.matmul(z[:], ns_sb[1][:, ml], s1_sb[:, sl],
                                     start=False, stop=False)
                    nc.tensor.matmul(z[:], ns_sb[2][:, ml], s2_sb[:, sl],
                                     start=False, stop=False)
                    nc.tensor.matmul(z[:], ns_sb[3][:, ml], ut_sb[:, sl],
                                     start=False, stop=True)
                    zb = z1pool.tile([128, 512], bf16, tag=f"z1b{hh}")
                    nc.scalar.activation(zb[:], z[:], AF.Relu,
                                         bias=b3_sb[:, hh:hh + 1])
                    zh.append(zb)
                o = pp1.tile([128, 512], f32, tag="misc")
                nc.tensor.matmul(o[:], w4_sb[:, 0:128], zh[0][:],
                                 start=True, stop=False)
                nc.tensor.matmul(o[:], w4_sb[:, 128:256], zh[1][:],
                                 start=False, stop=True)
                osb = opool.tile([128, 512], f32, tag="osb")
                nc.scalar.activation(osb[:], o[:], AF.Identity,
                                     bias=b4_sb[:, 0:1])
                nc.sync.dma_start(out_t.ap()[:, sl], osb[:])

            relu_acc = 0.0
            for b in range(NB):
                S = pp2.tile([128, H], f32, tag="S")
                for jj in range(0, NCHB, 2):
                    g2 = pp3.tile([128, 512], f32, tag="g")
                    for t in (0, 1):
                        ch = b * NCHB + jj + t
                        gs = g2[:, t * 256:(t + 1) * 256]
                        nc.tensor.matmul(gs, chunk_slice("xg", ch), w1a_sb[:],
                                         start=True, stop=False)
                        nc.tensor.matmul(gs, chunk_slice("ea", ch), w1b_sb[:],
                                         start=False, stop=not has_b1)
                        if has_b1:
                            nc.tensor.matmul(gs, ones_sb[0:1, :],
                                             b1r_sb[0:1, :],
                                             start=False, stop=True)
                    r2 = wpool.tile([128, 512], bf16, tag="r")
                    relu_acc += ACT_RELU_FRAC
                    if relu_acc >= 1.0:
                        relu_acc -= 1.0
                        nc.scalar.activation(r2[:], g2[:], AF.Relu)
                    else:
                        nc.vector.tensor_scalar(r2[:], g2[:], 0.0, None,
                                                ALU.max)
                    for t in (0, 1):
                        ch = b * NCHB + jj + t
                        nc.tensor.matmul(S[:], chunk_slice("oh", ch),
                                         r2[:, t * 256:(t + 1) * 256],
                                         start=(jj + t == 0),
                                         stop=(jj + t == NCHB - 1))
                # block tail
                sm = spool.tile([128, H], f32, tag="sm")
                nc.vector.tensor_scalar_mul(sm[:], S[:], inv_sb[:, b:b + 1])
                tr = pp1.tile([128, H], f32, tag="misc")
                nc.tensor.transpose(tr[:, 0:128], sm[:, 0:128], ident_sb[:])
                nc.tensor.transpose(tr[:, 128:256], sm[:, 128:256],
                                    ident_sb[:])
                nc.scalar.activation(s1_sb[:, b * 128:(b + 1) * 128],
                                     tr[:, 0:128], AF.Copy)
                nc.scalar.activation(s2_sb[:, b * 128:(b + 1) * 128],
                                     tr[:, 128:256], AF.Copy)
                if (b + 1) % 4 == 0:
                    node_group((b + 1) // 4 - 1)

    nc.compile()
    return nc


# ------------------------------------------------------------------- driver

def _install_ntff_hook():
    try:
        import antenv
        import concourse.bass_utils as bass_utils
        import trn_agent_boot.trn_boot as tb

        if "antenv.axon_hooks" in sys.modules:
            return
        mod = types.ModuleType("antenv.axon_hooks")
        holder = [None]
        mod.set_axon_ntff_profile_hook = lambda h: holder.__setitem__(0, h)
        mod.get_axon_ntff_profile_hook = lambda: holder[0]
        sys.modules["antenv.axon_hooks"] = mod
        antenv.axon_hooks = mod
        mod.set_axon_ntff_profile_hook(
            tb._ntff_profile_via_ctypes("/opt/axon/libaxon_pjrt.so"))
        bass_utils.upload_artifacts = lambda tmpdir: tmpdir
    except Exception:
        pass


def _run(inputs, trace=False, trace_cores=None):
    from concourse.bass_utils import run_bass_kernel_spmd

    if trace:
        _install_ntff_hook()
    meta, in_maps, dest_maps = _host_prep(inputs)
    key = tuple(sorted(meta.items()))
    if key not in _CACHE:
        _CACHE[key] = _build_program(meta)
    nc = _CACHE[key]
    res = run_bass_kernel_spmd(nc, in_maps, list(range(NCORES)), trace=trace,
                               trace_cores=trace_cores)
    N, FOUT = meta["N"], meta["FOUT"]
    out = np.zeros((N, FOUT), np.float32)
    for k in range(NCORES):
        o = res.results[k]["out_t"]
        dm = dest_maps[k]
        valid = dm >= 0
        out[dm[valid]] = o.T[valid]
    return out, res


def kernel(**inputs):
    return _run(inputs, trace=False)[0]
